# revision 28
# baseline (speedup 1.0000x reference)
"""Trainium2 Bass kernel for C2C attention.

Computes, for x:(B,C,T)=(32,64,30000) f32:
    desc = mean(x, axis=2)                       # (B,C)
    q = desc*Wq + bq ; k = desc*Wk + bk          # (B,C,D), D=64
    attn = softmax(q @ k^T / sqrt(D))            # (B,C,C)
    out = x + alpha * attn @ x

Sharding: pure data parallel over batch, 4 batches per core on 8 cores.
Each core stacks its 4 batches as 2 "pairs" of 128 SBUF partitions
(2 batches x 64 channels); a block-diagonal 128x128 stationary matrix
computes both batches of a pair in one matmul pass.

Transport is bf16 both ways (host rounds x to bf16; host expands the
bf16 result back to f32).  Residual + softmax normalization are folded
into the matmul pipeline:

    M^T = diag(sumexp/alpha) + exp(logits)^T    (stationary, bf16)
    out_row_c = (alpha/sumexp_c) * (M x)_c      (scale applied by the
                                                 PSUM->SBUF copy)

v2 schedule (vs the 105us baseline):
  - loads issue on GPSIMD/SWDGE (qPoolDynamic), stores on Sync/HWDGE
    (qSyncDynamicHW) so store packets round-robin with any still-draining
    load packets instead of queueing behind them
  - tail segments are small (4000/2000 cols) so the last fold before the
    attention chain is short
  - output copies alternate ACT/DVE per 1500-col group; pair1 fold work
    is interleaved into the DVE queue between copies, with GPSIMD
    prefolding the first two pair1 segments
  - stores are issued per segment as soon as its copies drain
"""

import os

import numpy as np
import ml_dtypes

import concourse.bass as bass
import concourse.tile as tile
from concourse import bacc, mybir
from concourse.bass_utils import run_bass_kernel_spmd


B, C, T, D = 32, 64, 30000, 64
N_CORES = 8
BPC = B // N_CORES          # batches per core = 4
PAIRS = BPC // 2            # 2
ROWS = BPC * C              # 256 rows of (row, T) per core
SEGS = (6000, 6000, 6000, 6000, 4000, 2000)   # per-pair column segments
NSEG = len(SEGS)
SEGOFF = tuple(sum(SEGS[:i]) for i in range(NSEG))
assert sum(SEGS) == T
CHUNK = 500                 # matmul moving free dim (<=512, one PSUM bank)
GCOLS = 1000                # columns per PSUM group (2 banks)

F32 = mybir.dt.float32
BF16 = mybir.dt.bfloat16
AX = mybir.AxisListType
AF = mybir.ActivationFunctionType
ALU = mybir.AluOpType

# groups per segment: (seg, [group widths])
def _seg_groups(seg):
    gs = []
    left = seg
    while left > 0:
        w = min(GCOLS, left)
        gs.append(w)
        left -= w
    return gs

# packed constants layout, one (128, 516) f32 block:
#   [:, 0:128]    identity(128)
#   [:, 128:129]  alpha broadcast
#   [0:2, 129:257]   [Wq/(8T); bq/8 | Wk/T; bk]  (stacked q|k weight rows)
#   [0:2, 257:385]   qk-matmul rhs init: row0 = 0 (sums placeholder), row1 = 1
#   [:, 385:513]  zeros -> attn scratch (off-diagonal blocks must stay 0)
#   [:, 513:514]  ones column (unused)
#   [:, 514:515]  1/alpha broadcast (unused)
#   [:, 515:516]  ln(1/alpha) broadcast (exp bias for the sumexp accum)
CONST_COLS = 516

# pair0 segments reduced by ACT (activation accum_out); rest on DVE
P0_ACT_SEGS = (1, 3)
# pair1 segments prefolded (levels 1+2) on GPSIMD; DVE finishes them
P1_GP_SEGS = (0, 1)

# ---- copy-engine schedule ----------------------------------------------
# pair0 has 30 groups (6+6+6+6+4+2 of 1000 cols).  ACT takes most pair0
# copies; the DVE stream owns a few early/late groups and interleaves
# pair1 fold work between them (a DVE-owned group right after a long
# fold would stall the PSUM ring, so DVE groups are spaced around the
# fold tokens).  AFTER_P0[g] = DVE ops to emit right after group g's
# copy: ('f', seg) full fold, ('fin', seg) finish of a GPSIMD prefold,
# ('sums',) the pair1 partials reduce.
# Only groups BEFORE the first fold token are DVE-owned: a DVE-owned
# group queued behind an arrival-gated fold would stall the PSUM ring
# (mm(g+3) waits on copy(g)) and starve ACT whenever pair1 loads land
# late.  ACT owns the whole tail instead.
DVE_GROUPS_P0 = {1, 3, 5, 7, 9, 11, 27, 29}
AFTER_P0 = {
    11: [('f', 2)],
    13: [('f', 3)],
    15: [('f', 4)],
    17: [('f', 5)],
    19: [('fin', 0)],
    20: [('fin', 1), ('sums',)],
}
# insert the pair1 attention chain after this pair0 group so PE/ACT
# reach it as soon as sums1 is ready instead of after all pair0 work
CHAIN_AT = 22
# pair1 segment emission order: a mid-size segment first so a store is
# ready quickly after the pair transition; the small one last so the
# final drain is short
P1_SEG_ORDER = (4, 0, 1, 2, 3, 5)
# pair1: alternate, DVE first (ACT finishes the attn chain first)
P1_ENGINES = ['d', 'a'] * 15


def build_bass() -> bass.Bass:
    nc = bacc.Bacc()

    x = nc.dram_tensor("x", [ROWS, T], BF16, kind="ExternalInput")
    out = nc.dram_tensor("out", [ROWS, T], BF16, kind="ExternalOutput")
    consts_d = nc.dram_tensor("consts", [128, CONST_COLS], F32,
                              kind="ExternalInput")

    with tile.TileContext(nc) as tc, \
            tc.tile_pool(name="consts", bufs=1) as consts, \
            tc.tile_pool(name="pairbuf", bufs=2) as pairbuf, \
            tc.tile_pool(name="fold", bufs=2) as fold, \
            tc.tile_pool(name="xsegs", bufs=PAIRS) as xsegs, \
            tc.tile_pool(name="psmm", bufs=3, space="PSUM") as psmm, \
            tc.tile_pool(name="pssm", bufs=2, space="PSUM") as pssm:

        cblk = consts.tile([128, CONST_COLS], F32)
        nc.sync.dma_start(out=cblk, in_=consts_d[:, :])
        ident = cblk[:, 0:128]
        alpha_bc = cblk[:, 128:129]
        wqk2 = cblk[0:2, 129:257]
        rhs_qk = cblk[0:2, 257:385]
        attn = cblk[:, 385:513]
        ln_invalpha = cblk[:, 515:516]
        scratch = consts.tile([128, 1], F32)
        # pre-load the ACT exp table off the critical path
        nc.scalar.activation(out=scratch, in_=alpha_bc, func=AF.Exp)
        # full-size scratch sink for ACT accumulate-reductions
        accsink = consts.tile([128, 6000], BF16)

        xs = [[None] * NSEG for _ in range(PAIRS)]
        partials = [None] * PAIRS
        lhsT = [None] * PAIRS
        arec = [None] * PAIRS
        sums = [None] * PAIRS
        sumexp = [None] * PAIRS
        diags = [None] * PAIRS
        gp_folds = {}

        def emit_load(p, s, eng=None):
            xt = xsegs.tile([128, SEGS[s]], BF16, tag=f"xseg{s}")
            xs[p][s] = xt
            (eng or nc.gpsimd).dma_start(
                out=xt,
                in_=x[p * 128:(p + 1) * 128,
                     SEGOFF[s]:SEGOFF[s] + SEGS[s]],
            )

        def emit_reduce_dve(p, s):
            # 3-level bf16 fold tree (2x DVE mode) + eighth-size reduce
            seg = SEGS[s]
            xt = xs[p][s]
            h = fold.tile([128, 3000], BF16, tag="h")
            h = h[:, 0:seg // 2]
            nc.vector.tensor_add(out=h, in0=xt[:, 0:seg // 2],
                                 in1=xt[:, seg // 2:seg])
            q = fold.tile([128, 1500], BF16, tag="q")
            q = q[:, 0:seg // 4]
            nc.vector.tensor_add(out=q, in0=h[:, 0:seg // 4],
                                 in1=h[:, seg // 4:seg // 2])
            w = fold.tile([128, 750], BF16, tag="w")
            w = w[:, 0:seg // 8]
            nc.vector.tensor_add(out=w, in0=q[:, 0:seg // 8],
                                 in1=q[:, seg // 8:seg // 4])
            nc.vector.reduce_sum(out=partials[p][:, s:s + 1], in_=w, axis=AX.X)

        def emit_reduce_act(p, s):
            # ACT-assisted reduction: copy into a scratch sink, accumulate
            # the row sum as a side effect
            seg = SEGS[s]
            nc.scalar.activation(out=accsink[:, 0:seg], in_=xs[p][s],
                                 func=AF.Copy,
                                 accum_out=partials[p][:, s:s + 1])

        def emit_gp_fold(p, s):
            # fold levels 1+2 on GPSIMD (SBUF->SBUF); DVE finishes with a
            # single fused quarter-size add+reduce
            seg = SEGS[s]
            xt = xs[p][s]
            gh = fold.tile([128, 3000], BF16, tag="gh")
            gh = gh[:, 0:seg // 2]
            nc.gpsimd.tensor_add(out=gh, in0=xt[:, 0:seg // 2],
                                 in1=xt[:, seg // 2:seg])
            gq = fold.tile([128, 1500], BF16, tag="gq")
            gq = gq[:, 0:seg // 4]
            nc.gpsimd.tensor_add(out=gq, in0=gh[:, 0:seg // 4],
                                 in1=gh[:, seg // 4:seg // 2])
            gp_folds[(p, s)] = gq

        def emit_dve_finish(p, s):
            seg = SEGS[s]
            gq = gp_folds[(p, s)]
            w = fold.tile([128, 750], BF16, tag="w")
            w = w[:, 0:seg // 8]
            nc.vector.tensor_add(out=w, in0=gq[:, 0:seg // 8],
                                 in1=gq[:, seg // 8:seg // 4])
            nc.vector.reduce_sum(out=partials[p][:, s:s + 1], in_=w, axis=AX.X)

        def alloc_partials(p):
            part = pairbuf.tile([128, NSEG], F32, tag="partial")
            partials[p] = part

        def emit_sums(p):
            sm = pairbuf.tile([128, 1], F32, tag="sums")
            nc.vector.reduce_sum(out=sm, in_=partials[p], axis=AX.X)
            sums[p] = sm

        def emit_smalls_head(p):
            # transpose total sums to a row: (1,128)
            srow_ps = pssm.tile([1, 128], F32, tag="ps_small")
            nc.tensor.transpose(out=srow_ps, in_=sums[p], identity=ident)
            nc.scalar.copy(out=rhs_qk[0:1, :], in_=srow_ps)
            # [qT; kT] = [wq'|wk']^T @ [sums_row; ones] : (128, 2C)
            qk_ps = pssm.tile([128, 2 * C], F32, tag="ps_small")
            nc.tensor.matmul(out=qk_ps, lhsT=wqk2, rhs=rhs_qk,
                             start=True, stop=True)
            qT = pairbuf.tile([D, 2 * C], F32, tag="qT")
            nc.scalar.copy(out=qT, in_=qk_ps[0:64, :])
            kT = pairbuf.tile([D, 2 * C], F32, tag="kT")
            nc.scalar.copy(out=kT, in_=qk_ps[64:128, :])
            # transposed logits lgT[e,c] = k_e . q_c, and plain logits
            # lg[c,e] (back-to-back PE ops, no extra hop)
            lg_ps = pssm.tile([128, 128], F32, tag="ps_small")
            nc.tensor.matmul(out=lg_ps, lhsT=qT, rhs=kT, start=True, stop=True)
            lgT_ps = pssm.tile([128, 128], F32, tag="ps_small")
            nc.tensor.matmul(out=lgT_ps, lhsT=kT, rhs=qT, start=True, stop=True)
            # exp of each diagonal block into the zeroed attn scratch; the
            # second (throwaway-output) exp over the plain logits accumulates
            # se_c = sum_e exp(lg[c,e] + ln(1/alpha)) = sumexp_c / alpha, so
            # reciprocal(se) = alpha/sumexp — exactly the copy scale
            se = pairbuf.tile([128, 1], F32, tag="sumexp")
            for h in range(2):
                r = slice(h * 64, h * 64 + 64)
                nc.scalar.activation(out=attn[r, r], in_=lgT_ps[r, r],
                                     func=AF.Exp)
                nc.scalar.activation(out=accsink[r, 0:64], in_=lg_ps[r, r],
                                     func=AF.Exp, bias=ln_invalpha[r, :],
                                     accum_out=se[r, :])
            sumexp[p] = se
            # diagonal part of the stationary matrix: diag(sumexp/alpha)
            diag = pairbuf.tile([128, 128], F32, tag="diag")
            nc.scalar.activation(out=diag, in_=ident, func=AF.Copy, scale=se)
            diags[p] = diag

        def emit_lt_dve(p):
            # stationary matrix M^T = diag(sumexp/alpha) + exp(logits)^T
            lt = pairbuf.tile([128, 128], BF16, tag="lhsT")
            nc.vector.tensor_add(out=lt, in0=diags[p], in1=attn)
            lhsT[p] = lt

        def emit_recip(p):
            # alpha/sumexp scale for the PSUM->SBUF copies
            ar = pairbuf.tile([128, 1], F32, tag="arec")
            nc.vector.reciprocal(out=ar, in_=sumexp[p])
            arec[p] = ar

        # per-pair group table: list of (seg, base-within-seg, width)
        groups = []
        for s in range(NSEG):
            base = 0
            for w in _seg_groups(SEGS[s]):
                groups.append((s, base, w))
                base += w
        NGRP = len(groups)        # 21
        last_group_of_seg = {}
        for gi, (s, base, w) in enumerate(groups):
            last_group_of_seg[s] = gi

        def emit_group_mm(p, gi):
            s, base, w = groups[gi]
            xt = xs[p][s]
            nch = (w + CHUNK - 1) // CHUNK
            mm = psmm.tile([128, 2, 512], F32, tag="mm")
            for j in range(nch):
                cw = min(CHUNK, w - j * CHUNK)
                nc.tensor.matmul(
                    out=mm[:, j, 0:cw],
                    lhsT=lhsT[p],
                    rhs=xt[:, base + j * CHUNK: base + j * CHUNK + cw],
                    start=True, stop=True,
                )
            return mm

        def emit_group_copy(p, gi, mm, eng):
            s, base, w = groups[gi]
            xt = xs[p][s]
            nch = (w + CHUNK - 1) // CHUNK
            if w == nch * CHUNK:
                dst = xt[:, base: base + w].rearrange(
                    "p (a c) -> p a c", a=nch)
                src = mm[:, 0:nch, 0:CHUNK]
            else:
                assert nch == 1
                dst = xt[:, base: base + w]
                src = mm[:, 0, 0:w]
            if eng == 'd':
                nc.vector.tensor_scalar(out=dst, in0=src,
                                        scalar1=arec[p], scalar2=None,
                                        op0=ALU.mult)
            else:
                nc.scalar.activation(out=dst, in_=src,
                                     func=AF.Copy, scale=arec[p])

        def emit_store_seg(p, s):
            nc.sync.dma_start(
                out=out[p * 128:(p + 1) * 128,
                        SEGOFF[s]:SEGOFF[s] + SEGS[s]],
                in_=xs[p][s],
            )

        # ---- schedule ----
        alloc_partials(0)
        alloc_partials(1)
        # first two pair0 loads on the (otherwise idle) sync HWDGE queue
        # for the fastest possible start; the rest on the gpsimd SWDGE
        # queue so stores never sit behind them
        emit_load(0, 0, eng=nc.sync)
        emit_load(0, 1, eng=nc.sync)
        for s in range(2, NSEG):
            emit_load(0, s)
        for s in range(NSEG):
            emit_load(1, s)
        # pair0 reductions in arrival order
        for s in range(NSEG):
            if s in P0_ACT_SEGS:
                emit_reduce_act(0, s)
            else:
                emit_reduce_dve(0, s)
        emit_sums(0)
        for s in P1_GP_SEGS:
            emit_gp_fold(1, s)
        emit_smalls_head(0)
        emit_lt_dve(0)
        emit_recip(0)
        # pair0 output stream with pair1 fold work woven into the DVE queue
        for gi in range(NGRP):
            mm = emit_group_mm(0, gi)
            emit_group_copy(0, gi, mm,
                            'd' if gi in DVE_GROUPS_P0 else 'a')
            s = groups[gi][0]
            if gi == last_group_of_seg[s]:
                emit_store_seg(0, s)
            for tok in AFTER_P0.get(gi, ()):
                if tok[0] == 'f':
                    emit_reduce_dve(1, tok[1])
                elif tok[0] == 'fin':
                    emit_dve_finish(1, tok[1])
                else:
                    emit_sums(1)
            if gi == CHAIN_AT:
                emit_smalls_head(1)
                emit_lt_dve(1)
                emit_recip(1)
        # pair1 in P1_SEG_ORDER, engines alternating along emission order
        p1_order = [gi for s in P1_SEG_ORDER
                    for gi in range(NGRP) if groups[gi][0] == s]
        for i, gi in enumerate(p1_order):
            mm = emit_group_mm(1, gi)
            emit_group_copy(1, gi, mm, P1_ENGINES[i])
            s = groups[gi][0]
            if gi == last_group_of_seg[s]:
                emit_store_seg(1, s)

    nc.compile()
    return nc


def _host_inputs(x, Wq, bq, Wk, bk, Wv, bv, alpha):
    """Build per-core input maps. Scale folding:
    logits[c,e] = (q[c]/8) . k[e],  q/8 = (Wq/(8T))*sums + bq/8, k = (Wk/T)*sums + bk
    """
    x = np.asarray(x, dtype=np.float32).astype(ml_dtypes.bfloat16)
    cb = np.zeros((128, CONST_COLS), dtype=np.float32)
    cb[:, 0:128] = np.eye(128, dtype=np.float32)
    cb[:, 128] = np.float32(alpha)
    cb[0, 129:193] = np.asarray(Wq)[:, 0] / (8.0 * T)
    cb[1, 129:193] = np.asarray(bq) / 8.0
    cb[0, 193:257] = np.asarray(Wk)[:, 0] / T
    cb[1, 193:257] = np.asarray(bk)
    cb[1, 257:385] = 1.0
    cb[:, 513] = 1.0
    cb[:, 514] = 1.0 / np.float32(alpha)
    cb[:, 515] = np.log(1.0 / np.float64(alpha)).astype(np.float32)
    in_maps = []
    for c in range(N_CORES):
        shard = x[c * BPC:(c + 1) * BPC].reshape(ROWS, T)
        in_maps.append({
            "x": np.ascontiguousarray(shard),
            "consts": cb,
        })
    return in_maps


def run(inputs: dict, trace: bool = False, tmpdir: str | None = None):
    nc = build_bass()
    in_maps = _host_inputs(**inputs)
    res = run_bass_kernel_spmd(
        nc, in_maps, core_ids=list(range(N_CORES)), trace=trace, tmpdir=tmpdir,
    )
    outs = [np.asarray(m["out"]).astype(np.float32).reshape(BPC, C, T)
            for m in res.results]
    full = np.concatenate(outs, axis=0)
    return full, res


def kernel(**inputs) -> np.ndarray:
    full, _ = run(inputs, trace=bool(os.environ.get("C2C_TRACE")))
    return full


if __name__ == "__main__":
    # quick single-core numerical check in CoreSim (+ timeline estimate)
    import sys
    from concourse import bass_interp
    from concourse.timeline_sim import TimelineSim

    nc = build_bass()
    print("timeline estimate:", TimelineSim(nc).simulate(), "ns")
    if "--timeline-only" in sys.argv:
        sys.exit(0)

    rng = np.random.default_rng(0)
    x = rng.standard_normal((BPC, C, T), dtype=np.float32)
    Wq = rng.standard_normal((D, 1)).astype(np.float32)
    bq = rng.standard_normal((D,)).astype(np.float32)
    Wk = rng.standard_normal((D, 1)).astype(np.float32)
    bk = rng.standard_normal((D,)).astype(np.float32)
    alpha = np.float32(0.5)

    sim = bass_interp.CoreSim(nc)
    im = _host_inputs(x=np.tile(x, (N_CORES, 1, 1)), Wq=Wq, bq=bq, Wk=Wk, bk=bk,
                      Wv=None, bv=None, alpha=alpha)[0]
    for k, v in im.items():
        sim.tensor(k)[:] = v
    sim.simulate()
    got = np.asarray(sim.tensor("out")).astype(np.float32).reshape(BPC, C, T)

    desc = x.mean(axis=2, keepdims=True)
    q = desc * Wq[:, 0] + bq
    k = desc * Wk[:, 0] + bk
    logits = np.einsum('bcd,bed->bce', q, k) / np.sqrt(D)
    m = logits.max(axis=-1, keepdims=True)
    e = np.exp(logits - m)
    attn = e / e.sum(axis=-1, keepdims=True)
    mixed = np.einsum('bce,bet->bct', attn, x)
    want = x + alpha * mixed
    err = np.abs(got - want)
    rel = np.linalg.norm(got - want) / np.linalg.norm(want)
    print("max abs err:", err.max(), "rel:", rel)


# revision 30
# speedup vs baseline: 1.3088x; 1.3088x over previous
"""Trainium2 Bass kernel for C2C attention.

Computes, for x:(B,C,T)=(32,64,30000) f32:
    desc = mean(x, axis=2)                       # (B,C)
    q = desc*Wq + bq ; k = desc*Wk + bk          # (B,C,D), D=64
    attn = softmax(q @ k^T / sqrt(D))            # (B,C,C)
    out = x + alpha * attn @ x

Sharding: pure data parallel over batch, 4 batches per core on 8 cores.
Each core stacks its 4 batches as 2 "pairs" of 128 SBUF partitions
(2 batches x 64 channels); a block-diagonal 128x128 stationary matrix
computes both batches of a pair in one matmul pass.

Transport is bf16 both ways (host rounds x to bf16; host expands the
bf16 result back to f32).  Residual + softmax normalization are folded
into the matmul pipeline:

    M^T = diag(sumexp/alpha) + exp(logits)^T    (stationary, bf16)
    out_row_c = (alpha/sumexp_c) * (M x)_c      (scale applied by the
                                                 PSUM->SBUF copy)

v2 schedule (vs the 105us baseline):
  - loads issue on GPSIMD/SWDGE (qPoolDynamic), stores on Sync/HWDGE
    (qSyncDynamicHW) so store packets round-robin with any still-draining
    load packets instead of queueing behind them
  - tail segments are small (4000/2000 cols) so the last fold before the
    attention chain is short
  - output copies alternate ACT/DVE per 1500-col group; pair1 fold work
    is interleaved into the DVE queue between copies, with GPSIMD
    prefolding the first two pair1 segments
  - stores are issued per segment as soon as its copies drain
"""

import os

import numpy as np
import ml_dtypes

import concourse.bass as bass
import concourse.tile as tile
from concourse import bacc, mybir
from concourse.bass_utils import run_bass_kernel_spmd


B, C, T, D = 32, 64, 30000, 64
N_CORES = 8
BPC = B // N_CORES          # batches per core = 4
PAIRS = BPC // 2            # 2
ROWS = BPC * C              # 256 rows of (row, T) per core
SEGS = (6000, 6000, 6000, 6000, 4000, 2000)   # per-pair column segments
NSEG = len(SEGS)
SEGOFF = tuple(sum(SEGS[:i]) for i in range(NSEG))
assert sum(SEGS) == T
CHUNK = 500                 # matmul moving free dim (<=512, one PSUM bank)
GCOLS = 1000                # columns per PSUM group (2 banks)

F32 = mybir.dt.float32
BF16 = mybir.dt.bfloat16
AX = mybir.AxisListType
AF = mybir.ActivationFunctionType
ALU = mybir.AluOpType

# groups per segment: (seg, [group widths])
def _seg_groups(seg):
    gs = []
    left = seg
    while left > 0:
        w = min(GCOLS, left)
        gs.append(w)
        left -= w
    return gs

# packed constants layout, one (128, 516) f32 block:
#   [:, 0:128]    identity(128)
#   [:, 128:129]  alpha broadcast
#   [0:2, 129:257]   [Wq/(8T); bq/8 | Wk/T; bk]  (stacked q|k weight rows)
#   [0:2, 257:385]   qk-matmul rhs init: row0 = 0 (sums placeholder), row1 = 1
#   [:, 385:513]  zeros -> attn scratch (off-diagonal blocks must stay 0)
#   [:, 513:514]  ones column (unused)
#   [:, 514:515]  1/alpha broadcast (unused)
#   [:, 515:516]  ln(1/alpha) broadcast (exp bias for the sumexp accum)
CONST_COLS = 516

# pair0 segments reduced by ACT (activation accum_out); rest on DVE
P0_ACT_SEGS = (1, 3)
# pair1 segments prefolded (levels 1+2) on GPSIMD; DVE finishes them
P1_GP_SEGS = (0, 1)

# ---- copy-engine schedule ----------------------------------------------
# pair0 has 30 groups (6+6+6+6+4+2 of 1000 cols).  ACT takes most pair0
# copies; the DVE stream owns a few early/late groups and interleaves
# pair1 fold work between them (a DVE-owned group right after a long
# fold would stall the PSUM ring, so DVE groups are spaced around the
# fold tokens).  AFTER_P0[g] = DVE ops to emit right after group g's
# copy: ('f', seg) full fold, ('fin', seg) finish of a GPSIMD prefold,
# ('sums',) the pair1 partials reduce.
# Only groups BEFORE the first fold token are DVE-owned: a DVE-owned
# group queued behind an arrival-gated fold would stall the PSUM ring
# (mm(g+3) waits on copy(g)) and starve ACT whenever pair1 loads land
# late.  ACT owns the whole tail instead.
DVE_GROUPS_P0 = {1, 3, 5, 7, 9, 11, 27, 29}
AFTER_P0 = {
    11: [('f', 2)],
    13: [('f', 3)],
    15: [('f', 4)],
    17: [('f', 5)],
    19: [('fin', 0)],
    20: [('fin', 1), ('sums',)],
}
# insert the pair1 attention chain after this pair0 group so PE/ACT
# reach it as soon as sums1 is ready instead of after all pair0 work
CHAIN_AT = 22
# pair1 segment emission order: a mid-size segment first so a store is
# ready quickly after the pair transition; the small one last so the
# final drain is short
P1_SEG_ORDER = (0, 1, 2, 3, 4, 5)
# pair1: alternate, DVE first (ACT finishes the attn chain first)
P1_ENGINES = ['d', 'a'] * 15


def build_bass() -> bass.Bass:
    nc = bacc.Bacc()

    x = nc.dram_tensor("x", [ROWS, T], BF16, kind="ExternalInput")
    out = nc.dram_tensor("out", [ROWS, T], BF16, kind="ExternalOutput")
    consts_d = nc.dram_tensor("consts", [128, CONST_COLS], F32,
                              kind="ExternalInput")

    with tile.TileContext(nc) as tc, \
            tc.tile_pool(name="consts", bufs=1) as consts, \
            tc.tile_pool(name="pairbuf", bufs=2) as pairbuf, \
            tc.tile_pool(name="fold", bufs=2) as fold, \
            tc.tile_pool(name="xsegs", bufs=PAIRS) as xsegs, \
            tc.tile_pool(name="psmm", bufs=3, space="PSUM") as psmm, \
            tc.tile_pool(name="pssm", bufs=2, space="PSUM") as pssm:

        cblk = consts.tile([128, CONST_COLS], F32)
        nc.sync.dma_start(out=cblk, in_=consts_d[:, :])
        ident = cblk[:, 0:128]
        alpha_bc = cblk[:, 128:129]
        wqk2 = cblk[0:2, 129:257]
        rhs_qk = cblk[0:2, 257:385]
        attn = cblk[:, 385:513]
        ln_invalpha = cblk[:, 515:516]
        scratch = consts.tile([128, 1], F32)
        # pre-load the ACT exp table off the critical path
        nc.scalar.activation(out=scratch, in_=alpha_bc, func=AF.Exp)
        # full-size scratch sink for ACT accumulate-reductions
        accsink = consts.tile([128, 6000], BF16)

        xs = [[None] * NSEG for _ in range(PAIRS)]
        partials = [None] * PAIRS
        lhsT = [None] * PAIRS
        arec = [None] * PAIRS
        sums = [None] * PAIRS
        sumexp = [None] * PAIRS
        diags = [None] * PAIRS
        gp_folds = {}

        def emit_load(p, s, eng=None):
            xt = xsegs.tile([128, SEGS[s]], BF16, tag=f"xseg{s}")
            xs[p][s] = xt
            (eng or nc.gpsimd).dma_start(
                out=xt,
                in_=x[p * 128:(p + 1) * 128,
                     SEGOFF[s]:SEGOFF[s] + SEGS[s]],
            )

        def emit_reduce_dve(p, s):
            # 3-level bf16 fold tree (2x DVE mode) + eighth-size reduce
            seg = SEGS[s]
            xt = xs[p][s]
            h = fold.tile([128, 3000], BF16, tag="h")
            h = h[:, 0:seg // 2]
            nc.vector.tensor_add(out=h, in0=xt[:, 0:seg // 2],
                                 in1=xt[:, seg // 2:seg])
            q = fold.tile([128, 1500], BF16, tag="q")
            q = q[:, 0:seg // 4]
            nc.vector.tensor_add(out=q, in0=h[:, 0:seg // 4],
                                 in1=h[:, seg // 4:seg // 2])
            w = fold.tile([128, 750], BF16, tag="w")
            w = w[:, 0:seg // 8]
            nc.vector.tensor_add(out=w, in0=q[:, 0:seg // 8],
                                 in1=q[:, seg // 8:seg // 4])
            nc.vector.reduce_sum(out=partials[p][:, s:s + 1], in_=w, axis=AX.X)

        def emit_reduce_act(p, s):
            # ACT-assisted reduction: copy into a scratch sink, accumulate
            # the row sum as a side effect
            seg = SEGS[s]
            nc.scalar.activation(out=accsink[:, 0:seg], in_=xs[p][s],
                                 func=AF.Copy,
                                 accum_out=partials[p][:, s:s + 1])

        def emit_gp_fold(p, s):
            # fold levels 1+2 on GPSIMD (SBUF->SBUF); DVE finishes with a
            # single fused quarter-size add+reduce
            seg = SEGS[s]
            xt = xs[p][s]
            gh = fold.tile([128, 3000], BF16, tag="gh")
            gh = gh[:, 0:seg // 2]
            nc.gpsimd.tensor_add(out=gh, in0=xt[:, 0:seg // 2],
                                 in1=xt[:, seg // 2:seg])
            gq = fold.tile([128, 1500], BF16, tag="gq")
            gq = gq[:, 0:seg // 4]
            nc.gpsimd.tensor_add(out=gq, in0=gh[:, 0:seg // 4],
                                 in1=gh[:, seg // 4:seg // 2])
            gp_folds[(p, s)] = gq

        def emit_dve_finish(p, s):
            seg = SEGS[s]
            gq = gp_folds[(p, s)]
            w = fold.tile([128, 750], BF16, tag="w")
            w = w[:, 0:seg // 8]
            nc.vector.tensor_add(out=w, in0=gq[:, 0:seg // 8],
                                 in1=gq[:, seg // 8:seg // 4])
            nc.vector.reduce_sum(out=partials[p][:, s:s + 1], in_=w, axis=AX.X)

        def alloc_partials(p):
            part = pairbuf.tile([128, NSEG], F32, tag="partial")
            partials[p] = part

        def emit_sums(p):
            sm = pairbuf.tile([128, 1], F32, tag="sums")
            nc.vector.reduce_sum(out=sm, in_=partials[p], axis=AX.X)
            sums[p] = sm

        def emit_smalls_head(p):
            # transpose total sums to a row: (1,128)
            srow_ps = pssm.tile([1, 128], F32, tag="ps_small")
            nc.tensor.transpose(out=srow_ps, in_=sums[p], identity=ident)
            nc.scalar.copy(out=rhs_qk[0:1, :], in_=srow_ps)
            # [qT; kT] = [wq'|wk']^T @ [sums_row; ones] : (128, 2C)
            qk_ps = pssm.tile([128, 2 * C], F32, tag="ps_small")
            nc.tensor.matmul(out=qk_ps, lhsT=wqk2, rhs=rhs_qk,
                             start=True, stop=True)
            qT = pairbuf.tile([D, 2 * C], F32, tag="qT")
            nc.scalar.copy(out=qT, in_=qk_ps[0:64, :])
            kT = pairbuf.tile([D, 2 * C], F32, tag="kT")
            nc.scalar.copy(out=kT, in_=qk_ps[64:128, :])
            # transposed logits lgT[e,c] = k_e . q_c, and plain logits
            # lg[c,e] (back-to-back PE ops, no extra hop)
            lg_ps = pssm.tile([128, 128], F32, tag="ps_small")
            nc.tensor.matmul(out=lg_ps, lhsT=qT, rhs=kT, start=True, stop=True)
            lgT_ps = pssm.tile([128, 128], F32, tag="ps_small")
            nc.tensor.matmul(out=lgT_ps, lhsT=kT, rhs=qT, start=True, stop=True)
            # exp of each diagonal block into the zeroed attn scratch; the
            # second (throwaway-output) exp over the plain logits accumulates
            # se_c = sum_e exp(lg[c,e] + ln(1/alpha)) = sumexp_c / alpha, so
            # reciprocal(se) = alpha/sumexp — exactly the copy scale
            se = pairbuf.tile([128, 1], F32, tag="sumexp")
            for h in range(2):
                r = slice(h * 64, h * 64 + 64)
                nc.scalar.activation(out=attn[r, r], in_=lgT_ps[r, r],
                                     func=AF.Exp)
                nc.scalar.activation(out=accsink[r, 0:64], in_=lg_ps[r, r],
                                     func=AF.Exp, bias=ln_invalpha[r, :],
                                     accum_out=se[r, :])
            sumexp[p] = se
            # diagonal part of the stationary matrix: diag(sumexp/alpha)
            diag = pairbuf.tile([128, 128], F32, tag="diag")
            nc.scalar.activation(out=diag, in_=ident, func=AF.Copy, scale=se)
            diags[p] = diag

        def emit_lt_dve(p):
            # stationary matrix M^T = diag(sumexp/alpha) + exp(logits)^T
            lt = pairbuf.tile([128, 128], BF16, tag="lhsT")
            nc.vector.tensor_add(out=lt, in0=diags[p], in1=attn)
            lhsT[p] = lt

        def emit_recip(p):
            # alpha/sumexp scale for the PSUM->SBUF copies
            ar = pairbuf.tile([128, 1], F32, tag="arec")
            nc.vector.reciprocal(out=ar, in_=sumexp[p])
            arec[p] = ar

        # per-pair group table: list of (seg, base-within-seg, width)
        groups = []
        for s in range(NSEG):
            base = 0
            for w in _seg_groups(SEGS[s]):
                groups.append((s, base, w))
                base += w
        NGRP = len(groups)        # 21
        last_group_of_seg = {}
        for gi, (s, base, w) in enumerate(groups):
            last_group_of_seg[s] = gi

        def emit_group_mm(p, gi):
            s, base, w = groups[gi]
            xt = xs[p][s]
            nch = (w + CHUNK - 1) // CHUNK
            mm = psmm.tile([128, 2, 512], F32, tag="mm")
            for j in range(nch):
                cw = min(CHUNK, w - j * CHUNK)
                nc.tensor.matmul(
                    out=mm[:, j, 0:cw],
                    lhsT=lhsT[p],
                    rhs=xt[:, base + j * CHUNK: base + j * CHUNK + cw],
                    start=True, stop=True,
                )
            return mm

        def emit_group_copy(p, gi, mm, eng):
            s, base, w = groups[gi]
            xt = xs[p][s]
            nch = (w + CHUNK - 1) // CHUNK
            if w == nch * CHUNK:
                dst = xt[:, base: base + w].rearrange(
                    "p (a c) -> p a c", a=nch)
                src = mm[:, 0:nch, 0:CHUNK]
            else:
                assert nch == 1
                dst = xt[:, base: base + w]
                src = mm[:, 0, 0:w]
            if eng == 'd':
                nc.vector.tensor_scalar(out=dst, in0=src,
                                        scalar1=arec[p], scalar2=None,
                                        op0=ALU.mult)
            else:
                nc.scalar.activation(out=dst, in_=src,
                                     func=AF.Copy, scale=arec[p])

        def emit_store_seg(p, s):
            nc.sync.dma_start(
                out=out[p * 128:(p + 1) * 128,
                        SEGOFF[s]:SEGOFF[s] + SEGS[s]],
                in_=xs[p][s],
            )

        # ---- schedule ----
        alloc_partials(0)
        alloc_partials(1)
        # all loads on the sync HWDGE queue, pair0 strictly first: the
        # wire then delivers pair0 at the full per-core rate, which sets
        # the serial head (mean -> attn -> first matmul).  Stores queue
        # behind the loads on the same ring; since copies (not stores)
        # bound the stream, the store backlog drains during and after it.
        for s in range(NSEG):
            emit_load(0, s, eng=nc.sync)
        for s in range(NSEG):
            emit_load(1, s, eng=nc.sync)
        # pair0 reductions in arrival order
        for s in range(NSEG):
            if s in P0_ACT_SEGS:
                emit_reduce_act(0, s)
            else:
                emit_reduce_dve(0, s)
        emit_sums(0)
        for s in P1_GP_SEGS:
            emit_gp_fold(1, s)
        emit_smalls_head(0)
        emit_lt_dve(0)
        emit_recip(0)
        # pair0 output stream with pair1 fold work woven into the DVE queue
        for gi in range(NGRP):
            mm = emit_group_mm(0, gi)
            emit_group_copy(0, gi, mm,
                            'd' if gi in DVE_GROUPS_P0 else 'a')
            s = groups[gi][0]
            if gi == last_group_of_seg[s]:
                emit_store_seg(0, s)
            for tok in AFTER_P0.get(gi, ()):
                if tok[0] == 'f':
                    emit_reduce_dve(1, tok[1])
                elif tok[0] == 'fin':
                    emit_dve_finish(1, tok[1])
                else:
                    emit_sums(1)
            if gi == CHAIN_AT:
                emit_smalls_head(1)
                emit_lt_dve(1)
                emit_recip(1)
        # pair1 in P1_SEG_ORDER, engines alternating along emission order
        p1_order = [gi for s in P1_SEG_ORDER
                    for gi in range(NGRP) if groups[gi][0] == s]
        for i, gi in enumerate(p1_order):
            mm = emit_group_mm(1, gi)
            emit_group_copy(1, gi, mm, P1_ENGINES[i])
            s = groups[gi][0]
            if gi == last_group_of_seg[s]:
                emit_store_seg(1, s)

    nc.compile()
    return nc


def _host_inputs(x, Wq, bq, Wk, bk, Wv, bv, alpha):
    """Build per-core input maps. Scale folding:
    logits[c,e] = (q[c]/8) . k[e],  q/8 = (Wq/(8T))*sums + bq/8, k = (Wk/T)*sums + bk
    """
    x = np.asarray(x, dtype=np.float32).astype(ml_dtypes.bfloat16)
    cb = np.zeros((128, CONST_COLS), dtype=np.float32)
    cb[:, 0:128] = np.eye(128, dtype=np.float32)
    cb[:, 128] = np.float32(alpha)
    cb[0, 129:193] = np.asarray(Wq)[:, 0] / (8.0 * T)
    cb[1, 129:193] = np.asarray(bq) / 8.0
    cb[0, 193:257] = np.asarray(Wk)[:, 0] / T
    cb[1, 193:257] = np.asarray(bk)
    cb[1, 257:385] = 1.0
    cb[:, 513] = 1.0
    cb[:, 514] = 1.0 / np.float32(alpha)
    cb[:, 515] = np.log(1.0 / np.float64(alpha)).astype(np.float32)
    in_maps = []
    for c in range(N_CORES):
        shard = x[c * BPC:(c + 1) * BPC].reshape(ROWS, T)
        in_maps.append({
            "x": np.ascontiguousarray(shard),
            "consts": cb,
        })
    return in_maps


def run(inputs: dict, trace: bool = False, tmpdir: str | None = None):
    nc = build_bass()
    in_maps = _host_inputs(**inputs)
    res = run_bass_kernel_spmd(
        nc, in_maps, core_ids=list(range(N_CORES)), trace=trace, tmpdir=tmpdir,
    )
    outs = [np.asarray(m["out"]).astype(np.float32).reshape(BPC, C, T)
            for m in res.results]
    full = np.concatenate(outs, axis=0)
    return full, res


def kernel(**inputs) -> np.ndarray:
    full, _ = run(inputs, trace=bool(os.environ.get("C2C_TRACE")))
    return full


if __name__ == "__main__":
    # quick single-core numerical check in CoreSim (+ timeline estimate)
    import sys
    from concourse import bass_interp
    from concourse.timeline_sim import TimelineSim

    nc = build_bass()
    print("timeline estimate:", TimelineSim(nc).simulate(), "ns")
    if "--timeline-only" in sys.argv:
        sys.exit(0)

    rng = np.random.default_rng(0)
    x = rng.standard_normal((BPC, C, T), dtype=np.float32)
    Wq = rng.standard_normal((D, 1)).astype(np.float32)
    bq = rng.standard_normal((D,)).astype(np.float32)
    Wk = rng.standard_normal((D, 1)).astype(np.float32)
    bk = rng.standard_normal((D,)).astype(np.float32)
    alpha = np.float32(0.5)

    sim = bass_interp.CoreSim(nc)
    im = _host_inputs(x=np.tile(x, (N_CORES, 1, 1)), Wq=Wq, bq=bq, Wk=Wk, bk=bk,
                      Wv=None, bv=None, alpha=alpha)[0]
    for k, v in im.items():
        sim.tensor(k)[:] = v
    sim.simulate()
    got = np.asarray(sim.tensor("out")).astype(np.float32).reshape(BPC, C, T)

    desc = x.mean(axis=2, keepdims=True)
    q = desc * Wq[:, 0] + bq
    k = desc * Wk[:, 0] + bk
    logits = np.einsum('bcd,bed->bce', q, k) / np.sqrt(D)
    m = logits.max(axis=-1, keepdims=True)
    e = np.exp(logits - m)
    attn = e / e.sum(axis=-1, keepdims=True)
    mixed = np.einsum('bce,bet->bct', attn, x)
    want = x + alpha * mixed
    err = np.abs(got - want)
    rel = np.linalg.norm(got - want) / np.linalg.norm(want)
    print("max abs err:", err.max(), "rel:", rel)
